# revision 7
# baseline (speedup 1.0000x reference)
"""Trainium2 Bass kernel for nn_MultiHeadAttention_71502615544564 (GNN
message-passing multi-head attention).

Math note: the reference computes
    out = segment_sum(v[dst] * attn_weights[..., None], dst)
Because v is indexed by the same dst as the segment reduction,
    out[n] = v[n] * (sum_e attn_weights[e]) = v[n] * s_n / (s_n + 1e-8)
where s_n = sum_exp[n].  For any node with in-degree >= 1, s_n is a sum
of exp values bounded below by exp(attn_min - attn_max) ~ 4e-2 for this
data, so s_n / (s_n + 1e-8) = 1 - O(3e-7): the whole attention pipeline
(q/k gathers, dots, exp, scatter) cancels out of the result.  Nodes with
in-degree 0 get out[n] = b_out exactly.  Hence

    out = x @ (Wv @ W_out) + (bv @ W_out + b_out),   in-deg 0 rows = b_out

which matches the reference to ~7e-7 in f32 (measured), ~4.6e-4 with
fp16 inputs/outputs (gate is 2e-2).  The device kernel is a node-sharded
GEMM with no gathers and no collectives; the in-degree-0 fixup is a
host-side bincount (this input has none).

Device layout: transposed GEMM.  outT[o, n] = Wf.T @ xT with the folded
weight Wf (bf16, packed as the head of the input tensor) as the
stationary (lhsT) operand, nodes on the 512-wide free axis -> 13
matmuls per core.  PSUM->SBUF copies (f32 -> fp16) alternate between
the Vector and Scalar engines; bias is added on the host during
unshard.  Input DMA is chunked (aligned to matmul chunks, small first
chunk) so compute chases the transfer; output DMA is 3 chunks so the
writeback overlaps the copy pipeline.

Sharding: node-parallel, 6250 nodes per core, each core fully computes
its own output rows.
"""

import sys

sys.path.insert(0, "/opt/trn_rl_repo")

import ml_dtypes
import numpy as np

import concourse.bacc as bacc
import concourse.mybir as mybir
import concourse.tile as tile
from concourse.bass_utils import run_bass_kernel_spmd

P = 128
N, DIM, H, HD = 50000, 128, 8, 16
NCORES = 8
NLOC = N // NCORES            # 6250 nodes per core
NKC = (NLOC + P - 1) // P     # 49 tiles
NKR = NKC * P                 # 6272 padded rows
XW = DIM + NKR                # packed input: [wf | xT]
MM = 512                      # matmul free-dim chunk (one PSUM bank)
# input DMA chunks (packed-column space, matmul-aligned), engine-tagged so
# issue cost splits across the two HWDGE queues (sync=SP, scalar=ACT)
IN_CHUNKS = [(0, 640, "sync"), (640, 1024, "scalar"), (1664, 1024, "sync"),
             (2688, 1024, "scalar"), (3712, 1024, "sync"),
             (4736, 1664, "scalar")]
# output DMA chunks (node-column space, matmul-aligned); gpsimd SWDGE queue
# for the even chunks keeps sync free and overlaps consecutive transfers
OUT_CHUNKS = [(0, 2048, "sync"), (2048, 2048, "gpsimd"),
              (4096, 1536, "sync"), (5632, 640, "gpsimd")]

F32 = mybir.dt.float32
FP16 = mybir.dt.float16
BF16 = mybir.dt.bfloat16
BF = ml_dtypes.bfloat16


def build_program():
    nc = bacc.Bacc("TRN2", target_bir_lowering=False, debug=False)

    xw = nc.dram_tensor("xw", [P, XW], BF16, kind="ExternalInput")
    out_loc = nc.dram_tensor("out_loc", [P, NKR], FP16, kind="ExternalOutput")

    with tile.TileContext(nc) as tc:
        with (
            tc.tile_pool(name="main", bufs=1) as pool,
            tc.tile_pool(name="ps", bufs=6, space="PSUM") as ps,
        ):
            xw_sb = pool.tile([P, XW], BF16)
            out_sb = pool.tile([P, NKR], FP16)

            for o0, sz, eng in IN_CHUNKS:
                getattr(nc, eng).dma_start(out=xw_sb[:, o0:o0 + sz],
                                           in_=xw[:, o0:o0 + sz])

            wf_sb = xw_sb[:, 0:DIM]
            for i, m0 in enumerate(range(0, NKR, MM)):
                msz = min(MM, NKR - m0)
                pt = ps.tile([P, MM], F32, tag="pt")
                nc.tensor.matmul(out=pt[:, :msz], lhsT=wf_sb,
                                 rhs=xw_sb[:, DIM + m0:DIM + m0 + msz],
                                 start=True, stop=True)
                if i % 2 == 0:
                    nc.vector.tensor_copy(out=out_sb[:, m0:m0 + msz],
                                          in_=pt[:, :msz])
                else:
                    nc.scalar.activation(out=out_sb[:, m0:m0 + msz],
                                         in_=pt[:, :msz],
                                         func=mybir.ActivationFunctionType.Copy)

            for o0, sz, eng in OUT_CHUNKS:
                getattr(nc, eng).dma_start(out=out_loc[:, o0:o0 + sz],
                                           in_=out_sb[:, o0:o0 + sz])

    nc.compile()
    return nc


def _prep(x, edge_index, W_qkv, b_qkv, W_out, b_out):
    x = np.asarray(x, np.float32)
    W_qkv = np.asarray(W_qkv, np.float32)
    b_qkv = np.asarray(b_qkv, np.float32)
    W_out = np.asarray(W_out, np.float32)
    b_out = np.asarray(b_out, np.float32)

    # v-projection columns of the packed qkv weight (per-head layout)
    hh = np.arange(H)[:, None]
    dd = np.arange(HD)[None, :]
    cols_v = (hh * 3 * HD + 2 * HD + dd).ravel()
    Wf = (W_qkv[:, cols_v] @ W_out).astype(BF)
    bf = b_qkv[cols_v] @ W_out + b_out  # f32, added on host

    in_maps = []
    for c in range(NCORES):
        xl = np.zeros((P, XW), BF)
        xl[:, :DIM] = Wf
        xl[:, DIM:DIM + NLOC] = x[c * NLOC:(c + 1) * NLOC].astype(BF).T
        in_maps.append({"xw": xl})
    return in_maps, bf


_PROG_CACHE = {}
TRACE = False
LAST_RESULT = None


def _install_ntff_hook():
    """Provide antenv.axon_hooks (absent in this image) so
    run_bass_kernel_spmd(trace=True) can NTFF-profile via libaxon."""
    import contextlib
    import ctypes
    import types

    if "antenv.axon_hooks" in sys.modules:
        return
    try:
        from antenv import axon_hooks  # noqa: F401
        return
    except ImportError:
        pass
    so_path = "/opt/axon/libaxon_pjrt.so"
    try:
        lib = ctypes.CDLL(so_path)
    except OSError:
        return
    if not hasattr(lib, "axon_start_nrt_profile"):
        return
    lib.axon_start_nrt_profile.argtypes = [
        ctypes.POINTER(ctypes.c_int64), ctypes.c_size_t]
    lib.axon_start_nrt_profile.restype = ctypes.c_int64
    lib.axon_stop_nrt_profile.argtypes = [ctypes.c_char_p]
    lib.axon_stop_nrt_profile.restype = ctypes.c_int64

    @contextlib.contextmanager
    def _hook(output_dir, device_ids):
        import jax
        jax.devices()
        if device_ids:
            ids = (ctypes.c_int64 * len(device_ids))(*device_ids)
            rc = lib.axon_start_nrt_profile(ids, len(device_ids))
        else:
            rc = lib.axon_start_nrt_profile(None, 0)
        if rc != 0:
            raise RuntimeError(f"axon_start_nrt_profile rc={rc}")
        try:
            yield
        finally:
            n = lib.axon_stop_nrt_profile(str(output_dir).encode())
            print(f"ntff profile: {n} file(s) -> {output_dir}", file=sys.stderr)

    _h = [_hook]
    m = types.ModuleType("antenv.axon_hooks")
    m.get_axon_ntff_profile_hook = lambda: _h[0]
    m.set_axon_ntff_profile_hook = lambda h: _h.__setitem__(0, h)
    sys.modules["antenv.axon_hooks"] = m
    import antenv
    antenv.axon_hooks = m


def kernel(x, edge_index, W_qkv, b_qkv, W_out, b_out):
    in_maps, bf = _prep(x, edge_index, W_qkv, b_qkv, W_out, b_out)
    if "p" not in _PROG_CACHE:
        _PROG_CACHE["p"] = build_program()
    nc = _PROG_CACHE["p"]
    if TRACE:
        _install_ntff_hook()
    res = run_bass_kernel_spmd(nc, in_maps, list(range(NCORES)), trace=TRACE)
    global LAST_RESULT
    LAST_RESULT = res
    out = np.empty((N, DIM), np.float32)
    for c in range(NCORES):
        o = np.asarray(res.results[c]["out_loc"])  # [DIM, NKR] fp16
        out[c * NLOC:(c + 1) * NLOC] = o[:, :NLOC].T.astype(np.float32) + bf

    # nodes with in-degree 0 receive no messages: out = b_out exactly
    dst = np.asarray(edge_index)[1].astype(np.int64)
    deg = np.bincount(dst, minlength=N)
    miss = deg == 0
    if miss.any():
        out[miss] = np.asarray(b_out, np.float32)
    return out


if __name__ == "__main__":
    rng = np.random.default_rng(0)
    x = rng.standard_normal((N, DIM)).astype(np.float32)
    ei = rng.integers(0, N, (2, 640000)).astype(np.int64)
    lim = 1.0 / np.sqrt(DIM)
    W_qkv = rng.uniform(-lim, lim, (DIM, 3 * DIM)).astype(np.float32)
    b_qkv = rng.uniform(-lim, lim, (3 * DIM,)).astype(np.float32)
    W_out = rng.uniform(-lim, lim, (DIM, DIM)).astype(np.float32)
    b_out = rng.uniform(-lim, lim, (DIM,)).astype(np.float32)
    out = kernel(x=x, edge_index=ei, W_qkv=W_qkv, b_qkv=b_qkv,
                 W_out=W_out, b_out=b_out)
    print("kernel output:", out.shape, out.dtype, np.abs(out).max())


# revision 8
# speedup vs baseline: 1.1837x; 1.1837x over previous
"""Trainium2 Bass kernel for nn_MultiHeadAttention_71502615544564 (GNN
message-passing multi-head attention).

Math note: the reference computes
    out = segment_sum(v[dst] * attn_weights[..., None], dst)
Because v is indexed by the same dst as the segment reduction,
    out[n] = v[n] * (sum_e attn_weights[e]) = v[n] * s_n / (s_n + 1e-8)
where s_n = sum_exp[n].  For any node with in-degree >= 1, s_n is a sum
of exp values bounded below by exp(attn_min - attn_max) ~ 4e-2 for this
data, so s_n / (s_n + 1e-8) = 1 - O(3e-7): the whole attention pipeline
(q/k gathers, dots, exp, scatter) cancels out of the result.  Nodes with
in-degree 0 get out[n] = b_out exactly.  Hence

    out = x @ (Wv @ W_out) + (bv @ W_out + b_out),   in-deg 0 rows = b_out

which matches the reference to ~7e-7 in f32 (measured), ~4.6e-4 with
fp16 inputs/outputs (gate is 2e-2).  The device kernel is a node-sharded
GEMM with no gathers and no collectives; the in-degree-0 fixup is a
host-side bincount (this input has none).

Device layout: transposed GEMM.  outT[o, n] = Wf.T @ xT with the folded
weight Wf (bf16, packed as the head of the input tensor) as the
stationary (lhsT) operand, nodes on the 512-wide free axis -> 13
matmuls per core.  PSUM->SBUF copies (f32 -> fp16) alternate between
the Vector and Scalar engines; bias is added on the host during
unshard.  Input DMA is chunked (aligned to matmul chunks, small first
chunk) so compute chases the transfer; output DMA is 3 chunks so the
writeback overlaps the copy pipeline.

Sharding: node-parallel, 6250 nodes per core, each core fully computes
its own output rows.
"""

import sys

sys.path.insert(0, "/opt/trn_rl_repo")

import ml_dtypes
import numpy as np

import concourse.bacc as bacc
import concourse.mybir as mybir
import concourse.tile as tile
from concourse.bass_utils import run_bass_kernel_spmd

P = 128
N, DIM, H, HD = 50000, 128, 8, 16
NCORES = 8
NLOC = N // NCORES            # 6250 nodes per core
NKC = (NLOC + P - 1) // P     # 49 tiles
NKR = NKC * P                 # 6272 padded rows
XW = DIM + NKR                # packed input: [wf | xT]
MM = 512                      # matmul free-dim chunk (one PSUM bank)
# input DMA chunks (packed-column space, matmul-aligned).  All on the sync
# HWDGE queue: a single ring drains strictly in order, so early chunks
# complete early and compute chases the transfer; splitting across queues
# makes the SDMA engines round-robin and delays every chunk (measured).
# Graded sizes: small first chunk starts the pipeline sooner.
IN_CHUNKS = [(0, 640, "sync"), (640, 1536, "sync"), (2176, 1536, "sync"),
             (3712, 1536, "sync"), (5248, 1152, "sync")]
# output DMA chunks (node-column space, matmul-aligned); the final chunk is
# small so the post-last-copy drain (issue + latency + transfer) is short
OUT_CHUNKS = [(0, 2048, "sync"), (2048, 2048, "sync"),
              (4096, 1536, "sync"), (5632, 640, "sync")]

F32 = mybir.dt.float32
FP16 = mybir.dt.float16
BF16 = mybir.dt.bfloat16
BF = ml_dtypes.bfloat16


def build_program():
    nc = bacc.Bacc("TRN2", target_bir_lowering=False, debug=False)

    xw = nc.dram_tensor("xw", [P, XW], BF16, kind="ExternalInput")
    out_loc = nc.dram_tensor("out_loc", [P, NKR], FP16, kind="ExternalOutput")

    with tile.TileContext(nc) as tc:
        with (
            tc.tile_pool(name="main", bufs=1) as pool,
            tc.tile_pool(name="ps", bufs=6, space="PSUM") as ps,
        ):
            xw_sb = pool.tile([P, XW], BF16)
            out_sb = pool.tile([P, NKR], FP16)

            for o0, sz, eng in IN_CHUNKS:
                getattr(nc, eng).dma_start(out=xw_sb[:, o0:o0 + sz],
                                           in_=xw[:, o0:o0 + sz])

            wf_sb = xw_sb[:, 0:DIM]
            for i, m0 in enumerate(range(0, NKR, MM)):
                msz = min(MM, NKR - m0)
                pt = ps.tile([P, MM], F32, tag="pt")
                nc.tensor.matmul(out=pt[:, :msz], lhsT=wf_sb,
                                 rhs=xw_sb[:, DIM + m0:DIM + m0 + msz],
                                 start=True, stop=True)
                if i % 2 == 0:
                    nc.vector.tensor_copy(out=out_sb[:, m0:m0 + msz],
                                          in_=pt[:, :msz])
                else:
                    nc.scalar.activation(out=out_sb[:, m0:m0 + msz],
                                         in_=pt[:, :msz],
                                         func=mybir.ActivationFunctionType.Copy)

            for o0, sz, eng in OUT_CHUNKS:
                getattr(nc, eng).dma_start(out=out_loc[:, o0:o0 + sz],
                                           in_=out_sb[:, o0:o0 + sz])

    nc.compile()
    return nc


def _prep(x, edge_index, W_qkv, b_qkv, W_out, b_out):
    x = np.asarray(x, np.float32)
    W_qkv = np.asarray(W_qkv, np.float32)
    b_qkv = np.asarray(b_qkv, np.float32)
    W_out = np.asarray(W_out, np.float32)
    b_out = np.asarray(b_out, np.float32)

    # v-projection columns of the packed qkv weight (per-head layout)
    hh = np.arange(H)[:, None]
    dd = np.arange(HD)[None, :]
    cols_v = (hh * 3 * HD + 2 * HD + dd).ravel()
    Wf = (W_qkv[:, cols_v] @ W_out).astype(BF)
    bf = b_qkv[cols_v] @ W_out + b_out  # f32, added on host

    in_maps = []
    for c in range(NCORES):
        xl = np.zeros((P, XW), BF)
        xl[:, :DIM] = Wf
        xl[:, DIM:DIM + NLOC] = x[c * NLOC:(c + 1) * NLOC].astype(BF).T
        in_maps.append({"xw": xl})
    return in_maps, bf


_PROG_CACHE = {}
TRACE = False
LAST_RESULT = None


def _install_ntff_hook():
    """Provide antenv.axon_hooks (absent in this image) so
    run_bass_kernel_spmd(trace=True) can NTFF-profile via libaxon."""
    import contextlib
    import ctypes
    import types

    if "antenv.axon_hooks" in sys.modules:
        return
    try:
        from antenv import axon_hooks  # noqa: F401
        return
    except ImportError:
        pass
    so_path = "/opt/axon/libaxon_pjrt.so"
    try:
        lib = ctypes.CDLL(so_path)
    except OSError:
        return
    if not hasattr(lib, "axon_start_nrt_profile"):
        return
    lib.axon_start_nrt_profile.argtypes = [
        ctypes.POINTER(ctypes.c_int64), ctypes.c_size_t]
    lib.axon_start_nrt_profile.restype = ctypes.c_int64
    lib.axon_stop_nrt_profile.argtypes = [ctypes.c_char_p]
    lib.axon_stop_nrt_profile.restype = ctypes.c_int64

    @contextlib.contextmanager
    def _hook(output_dir, device_ids):
        import jax
        jax.devices()
        if device_ids:
            ids = (ctypes.c_int64 * len(device_ids))(*device_ids)
            rc = lib.axon_start_nrt_profile(ids, len(device_ids))
        else:
            rc = lib.axon_start_nrt_profile(None, 0)
        if rc != 0:
            raise RuntimeError(f"axon_start_nrt_profile rc={rc}")
        try:
            yield
        finally:
            n = lib.axon_stop_nrt_profile(str(output_dir).encode())
            print(f"ntff profile: {n} file(s) -> {output_dir}", file=sys.stderr)

    _h = [_hook]
    m = types.ModuleType("antenv.axon_hooks")
    m.get_axon_ntff_profile_hook = lambda: _h[0]
    m.set_axon_ntff_profile_hook = lambda h: _h.__setitem__(0, h)
    sys.modules["antenv.axon_hooks"] = m
    import antenv
    antenv.axon_hooks = m


def kernel(x, edge_index, W_qkv, b_qkv, W_out, b_out):
    in_maps, bf = _prep(x, edge_index, W_qkv, b_qkv, W_out, b_out)
    if "p" not in _PROG_CACHE:
        _PROG_CACHE["p"] = build_program()
    nc = _PROG_CACHE["p"]
    if TRACE:
        _install_ntff_hook()
    res = run_bass_kernel_spmd(nc, in_maps, list(range(NCORES)), trace=TRACE)
    global LAST_RESULT
    LAST_RESULT = res
    out = np.empty((N, DIM), np.float32)
    for c in range(NCORES):
        o = np.asarray(res.results[c]["out_loc"])  # [DIM, NKR] fp16
        out[c * NLOC:(c + 1) * NLOC] = o[:, :NLOC].T.astype(np.float32) + bf

    # nodes with in-degree 0 receive no messages: out = b_out exactly
    dst = np.asarray(edge_index)[1].astype(np.int64)
    deg = np.bincount(dst, minlength=N)
    miss = deg == 0
    if miss.any():
        out[miss] = np.asarray(b_out, np.float32)
    return out


if __name__ == "__main__":
    rng = np.random.default_rng(0)
    x = rng.standard_normal((N, DIM)).astype(np.float32)
    ei = rng.integers(0, N, (2, 640000)).astype(np.int64)
    lim = 1.0 / np.sqrt(DIM)
    W_qkv = rng.uniform(-lim, lim, (DIM, 3 * DIM)).astype(np.float32)
    b_qkv = rng.uniform(-lim, lim, (3 * DIM,)).astype(np.float32)
    W_out = rng.uniform(-lim, lim, (DIM, DIM)).astype(np.float32)
    b_out = rng.uniform(-lim, lim, (DIM,)).astype(np.float32)
    out = kernel(x=x, edge_index=ei, W_qkv=W_qkv, b_qkv=b_qkv,
                 W_out=W_out, b_out=b_out)
    print("kernel output:", out.shape, out.dtype, np.abs(out).max())
